# revision 9
# baseline (speedup 1.0000x reference)
"""Trainium2 Bass kernel for nn_AuxLoss (aux CE loss + erf regularizer, segment-
mean over K=10 classes), data-parallel over 8 NeuronCores.

Math (per reference):
  f(u)      = 0.5 - 0.5*erf((-0.5 - u)/(sigma*sqrt2)) = 0.5 + 0.5*erf(sqrt2*u + sqrt2/2)
  row_reg_n = sum_d f(u[n,d])
  row_ce_n  = logsumexp(yg[n,:]) - yg[n, yhat[n]]
  per-class means over rows with yhat==k, averaged over present classes:
  out = aux + lmbd * reg

v3 design. Measured facts from the v1 (126.2 us) and v2 (130.8 us) traces:
the wire saturates at ~420-439 GB/s under BOTH a dual-ring and a single-ring
issue pattern, but with everything on the one sync HWDGE ring (v2), SDMA
engine 15 degrades to ~20.7 GB/s per packet (others 26.7) for the whole run;
a transfer's completion semaphore fires when the SLOWEST of the 16 engines
finishes, so sem-visible arrivals ran at ~6.4 us per 2 MB chunk instead of
5.0, the erf stream slipped behind, and the erf-gated u issues turned the
slip into wire starvation from 95 us on. v1 instead lost 17 us to the ACT
prologue: yg shared the wire with u until ~38 us, the first erf started at
43.7 us, and ACT finished its backlog long after the last byte. v3 keeps
v1's dual-ring issue pattern (no engine-15 pathology) and v2's consumption
ordering:
  - Mirrored half-chunks: u chunk ci's rows 0..31 ride the sync HWDGE ring,
    rows 32..63 the scalar HWDGE ring, at the same queue depth, so both
    halves land together and whole chunks complete every ~5 us in strict ci
    order. yg pieces head BOTH queues (sync 1,3,5,7 / scalar 0,2,4,6), so
    yg is fully landed by ~24 us and the exp/ln prologue is done by ~30 --
    erf c0 starts ~31 and the erf stream is land-paced ever after: erf(ci)
    ends ~= land(ci) + 3.7 us, i.e. the LAST byte + one tail piece.
  - Tail taper: erf pieces for c14 = 2 halves, c15 = 4 quarters (DMA'd as
    mirrored quarter-pairs), so the final erf work tracks the last landings
    at sub-microsecond granularity instead of paying a full 3.71 us chunk.
  - Scalar-ring issues are woven into the ACT stream where their waits are
    pre-satisfied (3 at t0, one after warm-exp, one after each early exp,
    then one after each erf); the sync ring takes its whole issue list at
    t0 and may stall freely on semaphore recycling / pool cap-gates
    (nothing else runs on it). u issues for chunks >= u_bufs are emitted
    after (and dep on) the erf whose completion frees their pool slot, so
    the counting cap-gate is satisfied by construction and slot requests
    stay in chunk order. No dependency ever lands ON a DMA instruction
    (that waits for the *transfer*). yhat rides the gpsimd SWDGE queue,
    casting i32->f32 in flight.
  - NO on-device collective: each core emits its raw [10, 67] f32 segment
    accumulator; the host sums the 8 cores and does the ~50-flop finish in
    numpy (the legitimate unshard step).
  - per 64-row chunk one bf16 work tile [128, 64, 67]:
      cols 0:64  erf(sqrt2*u + sqrt2/2)    (ACT, strided out; the 0.5+0.5*
                 affine is folded into the host fixup)
      col  64    picked = sum_c onehot*yg  (DVE row-reduce of onehot*raw yg)
      col  65    ones                      (counts)
      col  66    lse                       (DVE bf16 copy from the ln output)
  - PE: per 128-row group ldweights(onehot[128,10]) + matmul(work[128,67]),
    accumulating PSUM [10,67] over all 1024 groups; onehot lives in one big
    bf16 tile computed early from yhat via iota-compare.
"""

import math
import sys

if "/opt/trn_rl_repo" not in sys.path:
    sys.path.insert(0, "/opt/trn_rl_repo")

import numpy as np

N_CORES = 8
N_FULL = 1048576
C = 10
D = 64
P = 128
ROWS_PER_CORE = N_FULL // N_CORES  # 131072
SQ2 = math.sqrt(2.0)
W_COLS = D + 3  # erf block | picked | ones | lse
W_OUT = W_COLS

COL_PICK = D
COL_ONES = D + 1
COL_LSE = D + 2


def build(rows_per_core=ROWS_PER_CORE, w=64, n_half=8, u_bufs=6,
          w_bufs=3, yge_bufs=2):
    from concourse import bacc, mybir, tile

    f32 = mybir.dt.float32
    bf16 = mybir.dt.bfloat16
    i32 = mybir.dt.int32
    FT = mybir.ActivationFunctionType
    ALU = mybir.AluOpType
    AX = mybir.AxisListType

    rpp = rows_per_core // P  # rows per partition (1024)
    assert rpp * P == rows_per_core
    nch = rpp // w  # chunks (16)
    assert nch * w == rpp
    slab = rpp // 4  # onehot slab rows (256)
    half = rpp // n_half  # yg piece rows (128)
    assert half * n_half == rpp
    assert w <= half and half % w == 0

    h2 = w // 2
    q4 = w // 4

    # DMA piece row-cuts per chunk: [sync-ring pieces], [scalar-ring pieces].
    # Mirrored halves for every chunk; the last chunk lands as two mirrored
    # quarter-pairs so the final erf pieces track the last landings.
    def dma_cuts(ci):
        if ci == nch - 1:
            return [(0, q4), (2 * q4, 3 * q4)], [(q4, 2 * q4), (3 * q4, w)]
        return [(0, h2)], [(h2, w)]

    # erf piece row-cuts; chunk-granular until the tail taper.
    def erf_cuts(ci):
        if ci == nch - 2:
            return [(0, h2), (h2, w)]
        if ci == nch - 1:
            return [(0, q4), (q4, 2 * q4), (2 * q4, 3 * q4), (3 * q4, w)]
        return [(0, w)]

    nc = bacc.Bacc("TRN2", target_bir_lowering=False, debug=False, num_devices=N_CORES)

    yh_d = nc.dram_tensor("yhat", [rows_per_core], i32, kind="ExternalInput")
    yg_d = nc.dram_tensor("yg", [rows_per_core, C], f32, kind="ExternalInput")
    u_d = nc.dram_tensor("u_zg", [rows_per_core, D], f32, kind="ExternalInput")
    out_d = nc.dram_tensor("out", [C, W_OUT], f32, kind="ExternalOutput")

    u_v = u_d[:].rearrange("(p r) d -> p r d", p=P)
    yg_v = yg_d[:].rearrange("(p r) c -> p r c", p=P)
    yh_v = yh_d[:].rearrange("(p r) -> p r", p=P)

    from concourse.tile_rust import add_dep_helper

    def mk_chain(box, reason):
        def link(inst):
            raw = getattr(inst, "ins", inst)
            if box[0] is not None:
                add_dep_helper(raw, box[0], sync=True, reason=reason)
            box[0] = raw
            return inst
        return link

    sc_box = [None]
    sc_ordered = mk_chain(sc_box, "act order")
    dve_ordered = mk_chain([None], "dve order")
    gp_ordered = mk_chain([None], "gpsimd order")

    with tile.TileContext(nc) as tc:
        with (
            tc.tile_pool(name="const", bufs=1) as constp,
            tc.tile_pool(name="io", bufs=1) as iop,
            tc.tile_pool(name="work", bufs=1) as workp,
            tc.tile_pool(name="psum", bufs=1, space="PSUM") as psump,
        ):
            # u tiles created in ci order (slot = ci % u_bufs). Chunks
            # 0..u_bufs-1 use fresh slots and are issued at t0; chunk
            # k >= u_bufs is issued right after -- with a dep on -- erf
            # (k - u_bufs), the instruction whose completion frees its slot.
            u_ts = {ci: iop.tile([P, w, D], f32, name="u_t", bufs=u_bufs)
                    for ci in range(nch)}

            def issue_u(ci, ring, after=None):
                eng = nc.sync if ring == 0 else nc.scalar
                for r0, r1 in dma_cuts(ci)[ring]:
                    inst = eng.dma_start(
                        u_ts[ci][:, r0:r1, :],
                        u_v[:, ci * w + r0 : ci * w + r1, :],
                    )
                    if after is not None:
                        add_dep_helper(
                            getattr(inst, "ins", inst), after, sync=True,
                            reason="u slot freed by this erf",
                        )

            # t0, sync ring: its yg pieces then the first u_bufs chunks'
            # sync halves, in consumption order. The sync sequencer may
            # stall freely on semaphore recycling / cap-gates.
            yg_ts = {}

            def issue_yg(h, eng):
                yg_t = iop.tile([P, half, C], f32, name="yg_t", bufs=n_half)
                eng.dma_start(yg_t[:], yg_v[:, h * half : (h + 1) * half, :])
                yg_ts[h] = yg_t

            for h in (1, 3, 5, 7):
                issue_yg(h, nc.sync)
            for ci in range(u_bufs):
                issue_u(ci, ring=0)
            # t0, scalar ring: only 3 issues -- a 4th would stall on DMA
            # semaphore recycling ahead of the ACT compute stream.
            for h in (0, 2, 4):
                issue_yg(h, nc.scalar)
            # yhat via gpsimd SWDGE, casting i32 -> f32 in flight
            yh_f = constp.tile([P, rpp], f32)
            gp_ordered(nc.gpsimd.dma_start(yh_f[:], yh_v))

            # --- constants ---
            erf_bias = constp.tile([P, 1], f32)
            nc.vector.memset(erf_bias[:], 0.5 * SQ2)
            iota_f = constp.tile([P, 1, C], f32)
            gp_ordered(nc.gpsimd.iota(
                iota_f[:, 0, :], [[1, C]],
                channel_multiplier=0, allow_small_or_imprecise_dtypes=True,
            ))

            # warm the exp table while the first DMAs are in flight
            warm_act = constp.tile([1, 1], f32)
            nc.vector.memset(warm_act[:], 1.0)
            wa_o = constp.tile([1, 1], f32)
            sc_ordered(nc.scalar.activation(wa_o[:], warm_act[:], FT.Exp))
            issue_yg(6, nc.scalar)

            # --- ACT phase 1: exp per yg piece, then one ln. The DVE sumexp
            # reduces MUST be emitted before the ln: the Tile dep tracker
            # orders accesses by emission, so a read emitted before its
            # writers gets no dependency. The onehot for slab 0 is emitted
            # first so it heads the DVE chain. ---
            sume = constp.tile([P, rpp], f32)
            lse16 = constp.tile([P, rpp], f32)
            ohbig = constp.tile([P, rpp, C], bf16)

            def do_oneh(s):
                s0, s1 = s * slab, (s + 1) * slab
                dve_ordered(nc.vector.tensor_tensor(
                    ohbig[:, s0:s1, :],
                    yh_f[:, s0:s1].broadcast_to([P, slab, C]),
                    iota_f[:].broadcast_to([P, slab, C]),
                    ALU.is_equal,
                ))

            yge_ts = {}
            for h in range(n_half):
                yge = workp.tile([P, half, C], bf16, name="yge", bufs=yge_bufs)
                sc_ordered(nc.scalar.activation(yge[:], yg_ts[h][:], FT.Exp))
                yge_ts[h] = yge
                h0 = h * half
                dve_ordered(nc.vector.reduce_sum(
                    sume[:, h0 : h0 + half], yge[:], axis=AX.X
                ))
                # scalar-ring halves of the first u_bufs chunks, one issue
                # per exp so a semaphore-recycling stall never parks the ACT
                # stream for long
                if h < u_bufs:
                    issue_u(h, ring=1)
            sc_ordered(nc.scalar.activation(lse16[:], sume[:], FT.Ln))
            do_oneh(0)

            # --- ACT phase 2: the erf stream, land-paced ---
            work_ts = {}

            def do_erf(ci):
                work_t = workp.tile([P, w, W_COLS], bf16, name="work_t", bufs=w_bufs)
                u_t = u_ts.pop(ci)
                for r0, r1 in erf_cuts(ci):
                    sc_ordered(
                        nc.scalar.activation(
                            work_t[:, r0:r1, 0:D], u_t[:, r0:r1, :], FT.Erf,
                            bias=erf_bias[:], scale=SQ2,
                        )
                    )
                work_ts[ci] = work_t

            for ci in range(nch):
                do_erf(ci)
                nxt = ci + u_bufs
                if nxt < nch:
                    issue_u(nxt, ring=0, after=sc_box[0])
                    issue_u(nxt, ring=1, after=sc_box[0])

            # --- DVE chain continues: side cols c0..c3, oneh s1..s3, side
            # cols c4..c15, accS ---
            def do_side(ci):
                r0 = ci * w
                h = ci // (half // w)
                hr0 = (ci % (half // w)) * w
                pg_t = workp.tile([P, w, C], bf16, name="pg_t", bufs=1)
                dve_ordered(nc.vector.tensor_tensor(
                    pg_t[:], ohbig[:, r0 : r0 + w, :],
                    yg_ts[h][:, hr0 : hr0 + w, :], ALU.mult,
                ))
                with nc.allow_low_precision(reason="picked row has 1 nonzero"):
                    dve_ordered(nc.vector.reduce_sum(
                        work_ts[ci][:, :, COL_PICK], pg_t[:], axis=AX.X
                    ))
                dve_ordered(nc.vector.memset(work_ts[ci][:, :, COL_ONES], 1.0))
                dve_ordered(nc.vector.tensor_copy(
                    work_ts[ci][:, :, COL_LSE], lse16[:, r0 : r0 + w]
                ))

            chunks_per_slab = slab // w
            for ci in range(chunks_per_slab):
                do_side(ci)
            for s in range(1, 4):
                do_oneh(s)
            for ci in range(chunks_per_slab, nch):
                do_side(ci)

            # --- PE segment accumulation ---
            ps = psump.tile([C, W_COLS], f32)
            for ci in range(nch):
                r0 = ci * w
                work_t = work_ts.pop(ci)
                for g in range(w):
                    first = ci == 0 and g == 0
                    last = ci == nch - 1 and g == w - 1
                    nc.tensor.matmul(
                        ps[:], ohbig[:, r0 + g, :], work_t[:, g, :],
                        start=first, stop=last,
                    )

            # --- emit the raw accumulator; host finishes ---
            accS = constp.tile([C, W_OUT], f32)
            dve_ordered(nc.vector.tensor_copy(accS[:], ps[:]))
            nc.sync.dma_start(out_d[:], accS[:])

    nc.compile()
    return nc


_NC_CACHE = {}


def _get_nc(**kw):
    key = tuple(sorted(kw.items()))
    if key not in _NC_CACHE:
        _NC_CACHE[key] = build(**kw)
    return _NC_CACHE[key]


def make_in_maps(yhat, yg, u_zg, rows_per_core=ROWS_PER_CORE):
    yhat = np.ascontiguousarray(np.asarray(yhat).astype(np.int32))
    yg = np.ascontiguousarray(np.asarray(yg, dtype=np.float32))
    u_zg = np.ascontiguousarray(np.asarray(u_zg, dtype=np.float32))
    n = yhat.shape[0]
    assert n == rows_per_core * N_CORES
    in_maps = []
    for i in range(N_CORES):
        s = slice(i * rows_per_core, (i + 1) * rows_per_core)
        in_maps.append({"yhat": yhat[s], "yg": yg[s], "u_zg": u_zg[s]})
    return in_maps


def _finish(acc_sum, lmbd):
    """acc_sum: [C, W_OUT] f64 summed over cores. ~50 flops in numpy."""
    seg_erf = acc_sum[:, 0:D].sum(axis=1)
    seg_pick = acc_sum[:, COL_PICK]
    cnt = acc_sum[:, COL_ONES]
    seg_lse = acc_sum[:, COL_LSE]
    present = cnt > 0
    denom = np.where(present, cnt, 1.0)
    seg_reg = 0.5 * D * cnt + 0.5 * seg_erf
    reg_c = seg_reg / (denom * D)
    aux_c = (seg_lse - seg_pick) / denom
    n_unique = present.sum()
    reg = np.where(present, reg_c, 0.0).sum() / n_unique
    aux = np.where(present, aux_c, 0.0).sum() / n_unique
    return np.float32(aux + float(lmbd) * reg)


def run(yhat, yg, u_zg, lmbd, trace=False, rows_per_core=ROWS_PER_CORE, **kw):
    from concourse import bass_utils

    nc = _get_nc(rows_per_core=rows_per_core, **kw)
    in_maps = make_in_maps(yhat, yg, u_zg, rows_per_core)
    res = bass_utils.run_bass_kernel_spmd(
        nc, in_maps, core_ids=list(range(N_CORES)), trace=trace
    )
    acc = np.zeros((C, W_OUT), dtype=np.float64)
    for r in res.results:
        acc += np.asarray(r["out"], dtype=np.float64)
    val = _finish(acc, lmbd)
    return val, res


def kernel(yhat, yg, u_zg, lmbd):
    val, _ = run(yhat, yg, u_zg, lmbd)
    return np.asarray(val, dtype=np.float32).reshape(())


# revision 15
# speedup vs baseline: 1.1217x; 1.1217x over previous
"""Trainium2 Bass kernel for nn_AuxLoss (aux CE loss + erf regularizer, segment-
mean over K=10 classes), data-parallel over 8 NeuronCores.

Math (per reference):
  f(u)      = 0.5 - 0.5*erf((-0.5 - u)/(sigma*sqrt2)) = 0.5 + 0.5*erf(sqrt2*u + sqrt2/2)
  row_reg_n = sum_d f(u[n,d])
  row_ce_n  = logsumexp(yg[n,:]) - yg[n, yhat[n]]
  per-class means over rows with yhat==k, averaged over present classes:
  out = aux + lmbd * reg

v3 design. Measured facts from the v1 (126.2 us) and v2 (130.8 us) traces:
the wire saturates at ~420-439 GB/s under BOTH a dual-ring and a single-ring
issue pattern, but with everything on the one sync HWDGE ring (v2), SDMA
engine 15 degrades to ~20.7 GB/s per packet (others 26.7) for the whole run;
a transfer's completion semaphore fires when the SLOWEST of the 16 engines
finishes, so sem-visible arrivals ran at ~6.4 us per 2 MB chunk instead of
5.0, the erf stream slipped behind, and the erf-gated u issues turned the
slip into wire starvation from 95 us on. v1 instead lost 17 us to the ACT
prologue: yg shared the wire with u until ~38 us, the first erf started at
43.7 us, and ACT finished its backlog long after the last byte. v3 keeps
v1's dual-ring issue pattern (no engine-15 pathology) and v2's consumption
ordering:
  - Mirrored half-chunks: u chunk ci's rows 0..31 ride the sync HWDGE ring,
    rows 32..63 the scalar HWDGE ring, at the same queue depth, so both
    halves land together and whole chunks complete every ~5 us in strict ci
    order. yg pieces head BOTH queues (sync 1,3,5,7 / scalar 0,2,4,6), so
    yg is fully landed by ~24 us and the exp/ln prologue is done by ~30 --
    erf c0 starts ~31 and the erf stream is land-paced ever after: erf(ci)
    ends ~= land(ci) + 3.7 us, i.e. the LAST byte + one tail piece.
  - Tail taper: erf pieces for c14 = 2 halves, c15 = 4 quarters (DMA'd as
    mirrored quarter-pairs), so the final erf work tracks the last landings
    at sub-microsecond granularity instead of paying a full 3.71 us chunk.
  - Scalar-ring issues are woven into the ACT stream where their waits are
    pre-satisfied (3 at t0, one after warm-exp, one after each early exp,
    then one after each erf); the sync ring takes its whole issue list at
    t0 and may stall freely on semaphore recycling / pool cap-gates
    (nothing else runs on it). u issues for chunks >= u_bufs are emitted
    after (and dep on) the erf whose completion frees their pool slot, so
    the counting cap-gate is satisfied by construction and slot requests
    stay in chunk order. No dependency ever lands ON a DMA instruction
    (that waits for the *transfer*). yhat rides the gpsimd SWDGE queue,
    casting i32->f32 in flight.
  - NO on-device collective: each core emits its raw [10, 67] f32 segment
    accumulator; the host sums the 8 cores and does the ~50-flop finish in
    numpy (the legitimate unshard step).
  - per 64-row chunk one bf16 work tile [128, 64, 67]:
      cols 0:64  erf(sqrt2*u + sqrt2/2)    (ACT, strided out; the 0.5+0.5*
                 affine is folded into the host fixup)
      col  64    picked = sum_c onehot*yg  (DVE row-reduce of onehot*raw yg)
      col  65    ones                      (counts)
      col  66    lse                       (DVE bf16 copy from the ln output)
  - PE: per 128-row group ldweights(onehot[128,10]) + matmul(work[128,67]),
    accumulating PSUM [10,67] over all 1024 groups; onehot lives in one big
    bf16 tile computed early from yhat via iota-compare.
"""

import math
import sys

if "/opt/trn_rl_repo" not in sys.path:
    sys.path.insert(0, "/opt/trn_rl_repo")

import numpy as np

N_CORES = 8
N_FULL = 1048576
C = 10
D = 64
P = 128
ROWS_PER_CORE = N_FULL // N_CORES  # 131072
SQ2 = math.sqrt(2.0)
W_COLS = D + 3  # erf block | picked | ones | lse
W_OUT = W_COLS

COL_PICK = D
COL_ONES = D + 1
COL_LSE = D + 2


def build(rows_per_core=ROWS_PER_CORE, w=64, n_half=8, u_bufs=6,
          w_bufs=3, yge_bufs=2):
    from concourse import bacc, mybir, tile

    f32 = mybir.dt.float32
    bf16 = mybir.dt.bfloat16
    i32 = mybir.dt.int32
    FT = mybir.ActivationFunctionType
    ALU = mybir.AluOpType
    AX = mybir.AxisListType

    rpp = rows_per_core // P  # rows per partition (1024)
    assert rpp * P == rows_per_core
    nch = rpp // w  # chunks (16)
    assert nch * w == rpp
    slab = rpp // 4  # onehot slab rows (256)
    half = rpp // n_half  # yg piece rows (128)
    assert half * n_half == rpp
    assert w <= half and half % w == 0

    # DMA piece row-cuts per chunk; erf pieces use the SAME cuts so each erf
    # is gated only on its own piece's transfer. Full 64-row transfers move
    # in 16 KB-per-partition packets at full per-engine rate; the taper is
    # confined to the first chunk (early erf start) and the last three
    # (tail tracking), where the smaller-packet cost is negligible.
    def chunk_cuts(ci):
        if ci == 0 or ci == nch - 3:
            return [0, w // 2, w]
        if ci == nch - 2:
            return [0, w // 3, (2 * w) // 3, w]
        if ci == nch - 1:
            q = w // 4
            return [0, q, 2 * q, 3 * q, w]
        return [0, w]

    nc = bacc.Bacc("TRN2", target_bir_lowering=False, debug=False, num_devices=N_CORES)

    yh_d = nc.dram_tensor("yhat", [rows_per_core], i32, kind="ExternalInput")
    yg_d = nc.dram_tensor("yg", [rows_per_core, C], f32, kind="ExternalInput")
    u_d = nc.dram_tensor("u_zg", [rows_per_core, D], f32, kind="ExternalInput")
    out_d = nc.dram_tensor("out", [C, W_OUT], f32, kind="ExternalOutput")

    u_v = u_d[:].rearrange("(p r) d -> p r d", p=P)
    yg_v = yg_d[:].rearrange("(p r) c -> p r c", p=P)
    yh_v = yh_d[:].rearrange("(p r) -> p r", p=P)

    from concourse.tile_rust import add_dep_helper

    def mk_chain(box, reason):
        def link(inst):
            raw = getattr(inst, "ins", inst)
            if box[0] is not None:
                add_dep_helper(raw, box[0], sync=True, reason=reason)
            box[0] = raw
            return inst
        return link

    sc_box = [None]
    sc_ordered = mk_chain(sc_box, "act order")
    dve_ordered = mk_chain([None], "dve order")
    gp_ordered = mk_chain([None], "gpsimd order")

    with tile.TileContext(nc) as tc:
        with (
            tc.tile_pool(name="const", bufs=1) as constp,
            tc.tile_pool(name="io", bufs=1) as iop,
            tc.tile_pool(name="work", bufs=1) as workp,
            tc.tile_pool(name="psum", bufs=1, space="PSUM") as psump,
        ):
            # u tiles created in ci order (slot = ci % u_bufs). Chunks
            # 0..u_bufs-1 use fresh slots and are issued at t0; chunk
            # k >= u_bufs is issued right after -- with a dep on -- erf
            # (k - u_bufs), the instruction whose completion frees its slot.
            u_ts = {ci: iop.tile([P, w, D], f32, name="u_t", bufs=u_bufs)
                    for ci in range(nch)}

            def issue_u(ci, after=None):
                cuts = chunk_cuts(ci)
                for r0, r1 in zip(cuts, cuts[1:]):
                    inst = nc.sync.dma_start(
                        u_ts[ci][:, r0:r1, :],
                        u_v[:, ci * w + r0 : ci * w + r1, :],
                    )
                    if after is not None:
                        add_dep_helper(
                            getattr(inst, "ins", inst), after, sync=True,
                            reason="u slot freed by this erf",
                        )

            # t0: the whole stream prefix on the sync ring, in consumption
            # order: yhat (needed by the onehot ~30 us in), yg 0..7 (so the
            # exp/ln prologue finishes by ~30 us), then the first u_bufs
            # chunks. The sync sequencer may stall freely on semaphore
            # recycling / pool cap-gates -- nothing else runs on it. NO
            # SWDGE data traffic: the gpsimd descriptor rings contend for
            # the SBUF ports of SDMA engines 7/15, and a transfer's
            # completion sem fires only when the slowest engine finishes.
            yh_i = constp.tile([P, rpp], i32)
            nc.sync.dma_start(yh_i[:], yh_v)
            yg_ts = {}
            for h in range(n_half):
                yg_t = iop.tile([P, half, C], f32, name="yg_t", bufs=n_half)
                nc.sync.dma_start(yg_t[:], yg_v[:, h * half : (h + 1) * half, :])
                yg_ts[h] = yg_t
            for ci in range(u_bufs):
                issue_u(ci)

            # --- constants ---
            erf_bias = constp.tile([P, 1], f32)
            nc.vector.memset(erf_bias[:], 0.5 * SQ2)
            iota_f = constp.tile([P, 1, C], f32)
            gp_ordered(nc.gpsimd.iota(
                iota_f[:, 0, :], [[1, C]],
                channel_multiplier=0, allow_small_or_imprecise_dtypes=True,
            ))
            # yhat lands as i32 on the sync ring; one DVE cast replaces the
            # old SWDGE cast-in-flight
            yh_f = constp.tile([P, rpp], f32)

            # warm the exp table while the first DMAs are in flight
            warm_act = constp.tile([1, 1], f32)
            nc.vector.memset(warm_act[:], 1.0)
            wa_o = constp.tile([1, 1], f32)
            sc_ordered(nc.scalar.activation(wa_o[:], warm_act[:], FT.Exp))

            # --- ACT phase 1: exp per yg piece, then one ln. The DVE sumexp
            # reduces MUST be emitted before the ln: the Tile dep tracker
            # orders accesses by emission, so a read emitted before its
            # writers gets no dependency. The onehot for slab 0 is emitted
            # first so it heads the DVE chain. ---
            sume = constp.tile([P, rpp], f32)
            lse16 = constp.tile([P, rpp], f32)
            ohbig = constp.tile([P, rpp, C], bf16)
            dve_ordered(nc.vector.tensor_copy(yh_f[:], yh_i[:]))

            def do_oneh(s):
                s0, s1 = s * slab, (s + 1) * slab
                dve_ordered(nc.vector.tensor_tensor(
                    ohbig[:, s0:s1, :],
                    yh_f[:, s0:s1].broadcast_to([P, slab, C]),
                    iota_f[:].broadcast_to([P, slab, C]),
                    ALU.is_equal,
                ))

            yge_ts = {}
            for h in range(n_half):
                yge = workp.tile([P, half, C], bf16, name="yge", bufs=yge_bufs)
                sc_ordered(nc.scalar.activation(yge[:], yg_ts[h][:], FT.Exp))
                yge_ts[h] = yge
                h0 = h * half
                dve_ordered(nc.vector.reduce_sum(
                    sume[:, h0 : h0 + half], yge[:], axis=AX.X
                ))
            sc_ordered(nc.scalar.activation(lse16[:], sume[:], FT.Ln))
            do_oneh(0)

            # --- ACT phase 2: the erf stream, land-paced ---
            work_ts = {}

            def do_erf(ci):
                work_t = workp.tile([P, w, W_COLS], bf16, name="work_t", bufs=w_bufs)
                u_t = u_ts.pop(ci)
                cuts = chunk_cuts(ci)
                for r0, r1 in zip(cuts, cuts[1:]):
                    sc_ordered(
                        nc.scalar.activation(
                            work_t[:, r0:r1, 0:D], u_t[:, r0:r1, :], FT.Erf,
                            bias=erf_bias[:], scale=SQ2,
                        )
                    )
                work_ts[ci] = work_t

            for ci in range(nch):
                do_erf(ci)
                nxt = ci + u_bufs
                if nxt < nch:
                    issue_u(nxt, after=sc_box[0])

            # --- DVE chain continues: side cols c0..c3, oneh s1..s3, side
            # cols c4..c15, accS ---
            def do_side(ci):
                r0 = ci * w
                h = ci // (half // w)
                hr0 = (ci % (half // w)) * w
                pg_t = workp.tile([P, w, C], bf16, name="pg_t", bufs=1)
                dve_ordered(nc.vector.tensor_tensor(
                    pg_t[:], ohbig[:, r0 : r0 + w, :],
                    yg_ts[h][:, hr0 : hr0 + w, :], ALU.mult,
                ))
                with nc.allow_low_precision(reason="picked row has 1 nonzero"):
                    dve_ordered(nc.vector.reduce_sum(
                        work_ts[ci][:, :, COL_PICK], pg_t[:], axis=AX.X
                    ))
                dve_ordered(nc.vector.memset(work_ts[ci][:, :, COL_ONES], 1.0))
                dve_ordered(nc.vector.tensor_copy(
                    work_ts[ci][:, :, COL_LSE], lse16[:, r0 : r0 + w]
                ))

            chunks_per_slab = slab // w
            for ci in range(chunks_per_slab):
                do_side(ci)
            for s in range(1, 4):
                do_oneh(s)
            for ci in range(chunks_per_slab, nch):
                do_side(ci)

            # --- PE segment accumulation ---
            ps = psump.tile([C, W_COLS], f32)
            for ci in range(nch):
                r0 = ci * w
                work_t = work_ts.pop(ci)
                for g in range(w):
                    first = ci == 0 and g == 0
                    last = ci == nch - 1 and g == w - 1
                    nc.tensor.matmul(
                        ps[:], ohbig[:, r0 + g, :], work_t[:, g, :],
                        start=first, stop=last,
                    )

            # --- emit the raw accumulator; host finishes ---
            accS = constp.tile([C, W_OUT], f32)
            dve_ordered(nc.vector.tensor_copy(accS[:], ps[:]))
            nc.sync.dma_start(out_d[:], accS[:])

    nc.compile()
    return nc


_NC_CACHE = {}


def _get_nc(**kw):
    key = tuple(sorted(kw.items()))
    if key not in _NC_CACHE:
        _NC_CACHE[key] = build(**kw)
    return _NC_CACHE[key]


def make_in_maps(yhat, yg, u_zg, rows_per_core=ROWS_PER_CORE):
    yhat = np.ascontiguousarray(np.asarray(yhat).astype(np.int32))
    yg = np.ascontiguousarray(np.asarray(yg, dtype=np.float32))
    u_zg = np.ascontiguousarray(np.asarray(u_zg, dtype=np.float32))
    n = yhat.shape[0]
    assert n == rows_per_core * N_CORES
    in_maps = []
    for i in range(N_CORES):
        s = slice(i * rows_per_core, (i + 1) * rows_per_core)
        in_maps.append({"yhat": yhat[s], "yg": yg[s], "u_zg": u_zg[s]})
    return in_maps


def _finish(acc_sum, lmbd):
    """acc_sum: [C, W_OUT] f64 summed over cores. ~50 flops in numpy."""
    seg_erf = acc_sum[:, 0:D].sum(axis=1)
    seg_pick = acc_sum[:, COL_PICK]
    cnt = acc_sum[:, COL_ONES]
    seg_lse = acc_sum[:, COL_LSE]
    present = cnt > 0
    denom = np.where(present, cnt, 1.0)
    seg_reg = 0.5 * D * cnt + 0.5 * seg_erf
    reg_c = seg_reg / (denom * D)
    aux_c = (seg_lse - seg_pick) / denom
    n_unique = present.sum()
    reg = np.where(present, reg_c, 0.0).sum() / n_unique
    aux = np.where(present, aux_c, 0.0).sum() / n_unique
    return np.float32(aux + float(lmbd) * reg)


def run(yhat, yg, u_zg, lmbd, trace=False, rows_per_core=ROWS_PER_CORE, **kw):
    from concourse import bass_utils

    nc = _get_nc(rows_per_core=rows_per_core, **kw)
    in_maps = make_in_maps(yhat, yg, u_zg, rows_per_core)
    res = bass_utils.run_bass_kernel_spmd(
        nc, in_maps, core_ids=list(range(N_CORES)), trace=trace
    )
    acc = np.zeros((C, W_OUT), dtype=np.float64)
    for r in res.results:
        acc += np.asarray(r["out"], dtype=np.float64)
    val = _finish(acc, lmbd)
    return val, res


def kernel(yhat, yg, u_zg, lmbd):
    val, _ = run(yhat, yg, u_zg, lmbd)
    return np.asarray(val, dtype=np.float32).reshape(())


# revision 20
# speedup vs baseline: 1.2141x; 1.0823x over previous
"""Trainium2 Bass kernel for nn_AuxLoss (aux CE loss + erf regularizer, segment-
mean over K=10 classes), data-parallel over 8 NeuronCores.

Math (per reference):
  f(u)      = 0.5 - 0.5*erf((-0.5 - u)/(sigma*sqrt2)) = 0.5 + 0.5*erf(sqrt2*u + sqrt2/2)
  row_reg_n = sum_d f(u[n,d])
  row_ce_n  = logsumexp(yg[n,:]) - yg[n, yhat[n]]
  per-class means over rows with yhat==k, averaged over present classes:
  out = aux + lmbd * reg

v3 design. Measured facts from the v1 (126.2 us) and v2 (130.8 us) traces:
the wire saturates at ~420-439 GB/s under BOTH a dual-ring and a single-ring
issue pattern, but with everything on the one sync HWDGE ring (v2), SDMA
engine 15 degrades to ~20.7 GB/s per packet (others 26.7) for the whole run;
a transfer's completion semaphore fires when the SLOWEST of the 16 engines
finishes, so sem-visible arrivals ran at ~6.4 us per 2 MB chunk instead of
5.0, the erf stream slipped behind, and the erf-gated u issues turned the
slip into wire starvation from 95 us on. v1 instead lost 17 us to the ACT
prologue: yg shared the wire with u until ~38 us, the first erf started at
43.7 us, and ACT finished its backlog long after the last byte. v3 keeps
v1's dual-ring issue pattern (no engine-15 pathology) and v2's consumption
ordering:
  - Mirrored half-chunks: u chunk ci's rows 0..31 ride the sync HWDGE ring,
    rows 32..63 the scalar HWDGE ring, at the same queue depth, so both
    halves land together and whole chunks complete every ~5 us in strict ci
    order. yg pieces head BOTH queues (sync 1,3,5,7 / scalar 0,2,4,6), so
    yg is fully landed by ~24 us and the exp/ln prologue is done by ~30 --
    erf c0 starts ~31 and the erf stream is land-paced ever after: erf(ci)
    ends ~= land(ci) + 3.7 us, i.e. the LAST byte + one tail piece.
  - Tail taper: erf pieces for c14 = 2 halves, c15 = 4 quarters (DMA'd as
    mirrored quarter-pairs), so the final erf work tracks the last landings
    at sub-microsecond granularity instead of paying a full 3.71 us chunk.
  - Scalar-ring issues are woven into the ACT stream where their waits are
    pre-satisfied (3 at t0, one after warm-exp, one after each early exp,
    then one after each erf); the sync ring takes its whole issue list at
    t0 and may stall freely on semaphore recycling / pool cap-gates
    (nothing else runs on it). u issues for chunks >= u_bufs are emitted
    after (and dep on) the erf whose completion frees their pool slot, so
    the counting cap-gate is satisfied by construction and slot requests
    stay in chunk order. No dependency ever lands ON a DMA instruction
    (that waits for the *transfer*). yhat rides the gpsimd SWDGE queue,
    casting i32->f32 in flight.
  - NO on-device collective: each core emits its raw [10, 67] f32 segment
    accumulator; the host sums the 8 cores and does the ~50-flop finish in
    numpy (the legitimate unshard step).
  - per 64-row chunk one bf16 work tile [128, 64, 67]:
      cols 0:64  erf(sqrt2*u + sqrt2/2)    (ACT, strided out; the 0.5+0.5*
                 affine is folded into the host fixup)
      col  64    picked = sum_c onehot*yg  (DVE row-reduce of onehot*raw yg)
      col  65    ones                      (counts)
      col  66    lse                       (DVE bf16 copy from the ln output)
  - PE: per 128-row group ldweights(onehot[128,10]) + matmul(work[128,67]),
    accumulating PSUM [10,67] over all 1024 groups; onehot lives in one big
    bf16 tile computed early from yhat via iota-compare.
"""

import math
import sys

if "/opt/trn_rl_repo" not in sys.path:
    sys.path.insert(0, "/opt/trn_rl_repo")

import numpy as np

N_CORES = 8
N_FULL = 1048576
C = 10
D = 64
P = 128
ROWS_PER_CORE = N_FULL // N_CORES  # 131072
SQ2 = math.sqrt(2.0)
W_COLS = D + 3  # erf block | picked | ones | lse
W_OUT = W_COLS

COL_PICK = D
COL_ONES = D + 1
COL_LSE = D + 2


def build(rows_per_core=ROWS_PER_CORE, w=64, n_half=8, u_bufs=6,
          w_bufs=3, yge_bufs=2):
    from concourse import bacc, mybir, tile

    f32 = mybir.dt.float32
    bf16 = mybir.dt.bfloat16
    i32 = mybir.dt.int32
    FT = mybir.ActivationFunctionType
    ALU = mybir.AluOpType
    AX = mybir.AxisListType

    rpp = rows_per_core // P  # rows per partition (1024)
    assert rpp * P == rows_per_core
    nch = rpp // w  # chunks (16)
    assert nch * w == rpp
    slab = rpp // 4  # onehot slab rows (256)
    half = rpp // n_half  # yg piece rows (128)
    assert half * n_half == rpp
    assert w <= half and half % w == 0

    # DMA piece row-cuts per chunk; erf pieces use the SAME cuts so each erf
    # is gated only on its own piece's transfer. Full 64-row transfers move
    # in 16 KB-per-partition packets at full per-engine rate; the taper is
    # confined to the first chunk (early erf start) and the last four
    # (tail tracking), where the smaller-packet cost is negligible.
    def chunk_cuts(ci):
        if ci in (0, nch - 4, nch - 3):
            return [0, w // 2, w]
        if ci == nch - 2:
            return [0, w // 3, (2 * w) // 3, w]
        if ci == nch - 1:
            q = w // 4
            return [0, q, 2 * q, 3 * q, w]
        return [0, w]

    # Ring assignment: even chunks ride the sync HWDGE ring, odd chunks the
    # scalar ring -- EXCEPT the last two chunks, which both sit at the END
    # of the sync ring so they land serially (in erf order) after the
    # scalar ring has drained, instead of as a simultaneous final pair.
    def u_ring(ci):
        if ci >= nch - 2:
            return 0
        return 0 if ci % 2 == 0 else 1

    nc = bacc.Bacc("TRN2", target_bir_lowering=False, debug=False, num_devices=N_CORES)

    yh_d = nc.dram_tensor("yhat", [rows_per_core], i32, kind="ExternalInput")
    yg_d = nc.dram_tensor("yg", [rows_per_core, C], f32, kind="ExternalInput")
    u_d = nc.dram_tensor("u_zg", [rows_per_core, D], f32, kind="ExternalInput")
    out_d = nc.dram_tensor("out", [C, W_OUT], f32, kind="ExternalOutput")

    u_v = u_d[:].rearrange("(p r) d -> p r d", p=P)
    yg_v = yg_d[:].rearrange("(p r) c -> p r c", p=P)
    yh_v = yh_d[:].rearrange("(p r) -> p r", p=P)

    from concourse.tile_rust import add_dep_helper

    def mk_chain(box, reason):
        def link(inst):
            raw = getattr(inst, "ins", inst)
            if box[0] is not None:
                add_dep_helper(raw, box[0], sync=True, reason=reason)
            box[0] = raw
            return inst
        return link

    sc_box = [None]
    sc_ordered = mk_chain(sc_box, "act order")
    dve_ordered = mk_chain([None], "dve order")
    gp_ordered = mk_chain([None], "gpsimd order")

    with tile.TileContext(nc) as tc:
        with (
            tc.tile_pool(name="const", bufs=1) as constp,
            tc.tile_pool(name="io", bufs=1) as iop,
            tc.tile_pool(name="work", bufs=1) as workp,
            tc.tile_pool(name="psum", bufs=1, space="PSUM") as psump,
        ):
            # u tiles created in ci order (slot = ci % u_bufs). Chunks
            # 0..u_bufs-1 use fresh slots and are issued at t0; chunk
            # k >= u_bufs is issued right after -- with a dep on -- erf
            # (k - u_bufs), the instruction whose completion frees its slot.
            u_ts = {ci: iop.tile([P, w, D], f32, name="u_t", bufs=u_bufs)
                    for ci in range(nch)}

            def issue_u(ci, after=None):
                eng = nc.sync if u_ring(ci) == 0 else nc.scalar
                cuts = chunk_cuts(ci)
                for r0, r1 in zip(cuts, cuts[1:]):
                    inst = eng.dma_start(
                        u_ts[ci][:, r0:r1, :],
                        u_v[:, ci * w + r0 : ci * w + r1, :],
                    )
                    if after is not None:
                        add_dep_helper(
                            getattr(inst, "ins", inst), after, sync=True,
                            reason="u slot freed by this erf",
                        )

            # t0, sync ring: yg evens then the first sync-ring u chunks, in
            # consumption order. The sync sequencer may stall freely on
            # semaphore recycling / pool cap-gates -- nothing else runs on
            # it.
            yg_ts = {}

            def issue_yg(h, eng):
                yg_t = iop.tile([P, half, C], f32, name="yg_t", bufs=n_half)
                eng.dma_start(yg_t[:], yg_v[:, h * half : (h + 1) * half, :])
                yg_ts[h] = yg_t

            for h in (0, 2, 4, 6):
                issue_yg(h, nc.sync)
            for ci in (0, 2, 4):
                issue_u(ci)
            # t0, scalar ring: only 3 issues -- a 4th would stall on DMA
            # semaphore recycling ahead of the ACT compute stream.
            for h in (1, 3, 5):
                issue_yg(h, nc.scalar)
            # yhat via gpsimd SWDGE, casting i32 -> f32 in flight
            yh_f = constp.tile([P, rpp], f32)
            gp_ordered(nc.gpsimd.dma_start(yh_f[:], yh_v))

            # --- constants ---
            erf_bias = constp.tile([P, 1], f32)
            nc.vector.memset(erf_bias[:], 0.5 * SQ2)
            iota_f = constp.tile([P, 1, C], f32)
            gp_ordered(nc.gpsimd.iota(
                iota_f[:, 0, :], [[1, C]],
                channel_multiplier=0, allow_small_or_imprecise_dtypes=True,
            ))
            # warm the exp table while the first DMAs are in flight
            warm_act = constp.tile([1, 1], f32)
            nc.vector.memset(warm_act[:], 1.0)
            wa_o = constp.tile([1, 1], f32)
            sc_ordered(nc.scalar.activation(wa_o[:], warm_act[:], FT.Exp))
            # scalar ring continues: last yg piece + first odd u chunk
            issue_yg(7, nc.scalar)
            issue_u(1)

            # --- ACT phase 1: exp per yg piece, then one ln. The DVE sumexp
            # reduces MUST be emitted before the ln: the Tile dep tracker
            # orders accesses by emission, so a read emitted before its
            # writers gets no dependency. The onehot for slab 0 is emitted
            # first so it heads the DVE chain. ---
            sume = constp.tile([P, rpp], f32)
            lse16 = constp.tile([P, rpp], f32)
            ohbig = constp.tile([P, rpp, C], bf16)

            def do_oneh(s):
                s0, s1 = s * slab, (s + 1) * slab
                dve_ordered(nc.vector.tensor_tensor(
                    ohbig[:, s0:s1, :],
                    yh_f[:, s0:s1].broadcast_to([P, slab, C]),
                    iota_f[:].broadcast_to([P, slab, C]),
                    ALU.is_equal,
                ))

            yge_ts = {}
            for h in range(n_half):
                yge = workp.tile([P, half, C], bf16, name="yge", bufs=yge_bufs)
                sc_ordered(nc.scalar.activation(yge[:], yg_ts[h][:], FT.Exp))
                yge_ts[h] = yge
                h0 = h * half
                dve_ordered(nc.vector.reduce_sum(
                    sume[:, h0 : h0 + half], yge[:], axis=AX.X
                ))
                # remaining fresh-slot odd chunks, one issue per early exp
                # so a semaphore-recycling stall never parks the ACT stream
                if h == 0:
                    issue_u(3)
                elif h == 2:
                    issue_u(5)
            sc_ordered(nc.scalar.activation(lse16[:], sume[:], FT.Ln))
            do_oneh(0)

            # --- ACT phase 2: the erf stream, land-paced ---
            work_ts = {}

            def do_erf(ci):
                work_t = workp.tile([P, w, W_COLS], bf16, name="work_t", bufs=w_bufs)
                u_t = u_ts.pop(ci)
                cuts = chunk_cuts(ci)
                for r0, r1 in zip(cuts, cuts[1:]):
                    sc_ordered(
                        nc.scalar.activation(
                            work_t[:, r0:r1, 0:D], u_t[:, r0:r1, :], FT.Erf,
                            bias=erf_bias[:], scale=SQ2,
                        )
                    )
                work_ts[ci] = work_t

            for ci in range(nch):
                do_erf(ci)
                nxt = ci + u_bufs
                if nxt < nch:
                    issue_u(nxt, after=sc_box[0])

            # --- DVE chain continues: side cols c0..c3, oneh s1..s3, side
            # cols c4..c15, accS ---
            def do_side(ci):
                r0 = ci * w
                h = ci // (half // w)
                hr0 = (ci % (half // w)) * w
                pg_t = workp.tile([P, w, C], bf16, name="pg_t", bufs=1)
                dve_ordered(nc.vector.tensor_tensor(
                    pg_t[:], ohbig[:, r0 : r0 + w, :],
                    yg_ts[h][:, hr0 : hr0 + w, :], ALU.mult,
                ))
                with nc.allow_low_precision(reason="picked row has 1 nonzero"):
                    dve_ordered(nc.vector.reduce_sum(
                        work_ts[ci][:, :, COL_PICK], pg_t[:], axis=AX.X
                    ))
                dve_ordered(nc.vector.memset(work_ts[ci][:, :, COL_ONES], 1.0))
                dve_ordered(nc.vector.tensor_copy(
                    work_ts[ci][:, :, COL_LSE], lse16[:, r0 : r0 + w]
                ))

            chunks_per_slab = slab // w
            for ci in range(chunks_per_slab):
                do_side(ci)
            for s in range(1, 4):
                do_oneh(s)
            for ci in range(chunks_per_slab, nch):
                do_side(ci)

            # --- PE segment accumulation ---
            ps = psump.tile([C, W_COLS], f32)
            for ci in range(nch):
                r0 = ci * w
                work_t = work_ts.pop(ci)
                for g in range(w):
                    first = ci == 0 and g == 0
                    last = ci == nch - 1 and g == w - 1
                    nc.tensor.matmul(
                        ps[:], ohbig[:, r0 + g, :], work_t[:, g, :],
                        start=first, stop=last,
                    )

            # --- emit the raw accumulator; host finishes ---
            accS = constp.tile([C, W_OUT], f32)
            dve_ordered(nc.vector.tensor_copy(accS[:], ps[:]))
            nc.sync.dma_start(out_d[:], accS[:])

    nc.compile()
    return nc


_NC_CACHE = {}


def _get_nc(**kw):
    key = tuple(sorted(kw.items()))
    if key not in _NC_CACHE:
        _NC_CACHE[key] = build(**kw)
    return _NC_CACHE[key]


def make_in_maps(yhat, yg, u_zg, rows_per_core=ROWS_PER_CORE):
    yhat = np.ascontiguousarray(np.asarray(yhat).astype(np.int32))
    yg = np.ascontiguousarray(np.asarray(yg, dtype=np.float32))
    u_zg = np.ascontiguousarray(np.asarray(u_zg, dtype=np.float32))
    n = yhat.shape[0]
    assert n == rows_per_core * N_CORES
    in_maps = []
    for i in range(N_CORES):
        s = slice(i * rows_per_core, (i + 1) * rows_per_core)
        in_maps.append({"yhat": yhat[s], "yg": yg[s], "u_zg": u_zg[s]})
    return in_maps


def _finish(acc_sum, lmbd):
    """acc_sum: [C, W_OUT] f64 summed over cores. ~50 flops in numpy."""
    seg_erf = acc_sum[:, 0:D].sum(axis=1)
    seg_pick = acc_sum[:, COL_PICK]
    cnt = acc_sum[:, COL_ONES]
    seg_lse = acc_sum[:, COL_LSE]
    present = cnt > 0
    denom = np.where(present, cnt, 1.0)
    seg_reg = 0.5 * D * cnt + 0.5 * seg_erf
    reg_c = seg_reg / (denom * D)
    aux_c = (seg_lse - seg_pick) / denom
    n_unique = present.sum()
    reg = np.where(present, reg_c, 0.0).sum() / n_unique
    aux = np.where(present, aux_c, 0.0).sum() / n_unique
    return np.float32(aux + float(lmbd) * reg)


def run(yhat, yg, u_zg, lmbd, trace=False, rows_per_core=ROWS_PER_CORE, **kw):
    from concourse import bass_utils

    nc = _get_nc(rows_per_core=rows_per_core, **kw)
    in_maps = make_in_maps(yhat, yg, u_zg, rows_per_core)
    res = bass_utils.run_bass_kernel_spmd(
        nc, in_maps, core_ids=list(range(N_CORES)), trace=trace
    )
    acc = np.zeros((C, W_OUT), dtype=np.float64)
    for r in res.results:
        acc += np.asarray(r["out"], dtype=np.float64)
    val = _finish(acc, lmbd)
    return val, res


def kernel(yhat, yg, u_zg, lmbd):
    val, _ = run(yhat, yg, u_zg, lmbd)
    return np.asarray(val, dtype=np.float32).reshape(())
